# revision 26
# baseline (speedup 1.0000x reference)
"""Masked multi-head attention block (B=4, N=1024, D=1024, H=16, DH=64) on 8
Trainium2 NeuronCores.

Sharding: core (b, g) = 2*b + g handles batch b and head-group g (8 of 16
heads). Each core computes qkv projections for its heads, attention, and its
partial output projection; the host sums the two head-group partials per batch.

Mask handling: tokens with mask==0 neither attend nor are attended to (the
reference sets fully-masked rows to zero and -inf-masks columns). The host
gathers only the valid tokens per batch (padded to a multiple of 128 with
key-bias -30000 on the pad), so the device computes a dense unmasked attention
over ~half the sequence; invalid token rows of the output are b_out.

Device layout (per core, Vp = padded valid token count):
  xT   [D, Vp]      gathered tokens, transposed (host prep)
  qkT  [1024, Vp]   = [Q^T; K^T] for 8 heads, via lhsT=wqk chunks, rhs=xT
  V'   [Vp, 8, 65]  values per head + ones column (softmax denominator)
  S^T  [Vp, Vp]/head = K_chunk @ Q^T; exp on ACT with per-partition pad bias
  O^T  [65, Vp]/head = V'^T @ P^T accumulated over key chunks (row 64 = denom)
  A^T  [64, Vp]/head = O^T * (1/denom) broadcast (DRAM-bounce broadcast DMA)
  y    [Vp, D]      = sum_h A_h @ w_out_h  (K=64 accumulating matmuls)
"""
import json
import os
import sys

import numpy as np

sys.path.insert(0, "/opt/trn_rl_repo")

import concourse.bass as bass
import concourse.mybir as mybir
from concourse.tile import TileContext
from concourse import bass_utils

F32 = mybir.dt.float32
F32R = mybir.dt.float32r
BF16 = mybir.dt.bfloat16
AF = mybir.ActivationFunctionType

B, N, D, H, DH = 4, 1024, 1024, 16, 64
NCORES = 8
PAD_BIAS = -30000.0


def _install_patches():
    """The walrus build in this container accepts only one semaphore wait per
    instruction; hoist extra waits onto same-engine NoOps in the BIR json."""
    if getattr(bass.Bass, "_split_waits_patched", False):
        return
    orig = bass.Bass.to_json_bytes

    def to_json_bytes_split(self, *a, **k):
        j = json.loads(orig(self, *a, **k))
        for fn in j.get("functions", []):
            for bb in fn.get("blocks", []):
                out = []
                for ins in bb.get("instructions", []):
                    si = ins.get("sync_info") or {}
                    waits = si.get("on_wait") or []
                    if len(waits) > 1:
                        for i, w in enumerate(waits[:-1]):
                            out.append({
                                "debug": ins.get("debug", 0),
                                "engine": ins["engine"],
                                "ins": [],
                                "name": f"{ins['name']}_sw{i}",
                                "opcode": "NoOp",
                                "outs": [],
                                "text_hint": "splitw",
                                "sync_info": {"on_update": [], "on_wait": [w]},
                            })
                        si["on_wait"] = [waits[-1]]
                    out.append(ins)
                bb["instructions"] = out
        return json.dumps(j).encode()

    bass.Bass.to_json_bytes = to_json_bytes_split

    def _drain_and_barrier(self, tick_clock, wait_clock):
        import re as _re
        import bass_rust as _br
        from concourse.vector_clock import ScopedClock as _SC
        gc = tick_clock.global_clock
        comps = eval(_re.match(r"VectorClock\((\[.*\])\)", repr(gc)).group(1))
        for i, v in enumerate(comps):
            if v <= 0:
                continue
            sub = [0] * len(comps)
            sub[i] = v
            nop = self.nc.sync.nop(nofuse=True, hint="final_wait")
            wait_clock.add_sem_waits(nop.ins, _SC({None: _br.VectorClock(sub)}))
        self.nc.sync.drain()
        self.nc.all_engine_barrier()
        assert self.sems is not None
        popped = self.nc._tile_sem_poison_stack.pop()
        assert popped is self._sem_poison
        self.nc.clear_and_free_semaphores(list(self.sems.allocated().values()))

    TileContext._drain_and_barrier = _drain_and_barrier
    bass.Bass._split_waits_patched = True


def _build_program(Vp, Vq):
    KC = Vp // 128
    # key-side and query-side column slices: each slice gets its own PSUM
    # bank (matmul output must not cross a bank) and stays >= 256 wide.
    W = Vp if Vp <= 512 else Vp // 2
    QS = [(i * W, W) for i in range(Vp // W)]
    NQ = len(QS)
    Wq = Vq if Vq <= 512 else Vq // 2
    QSq = [(i * Wq, Wq) for i in range(Vq // Wq)]
    NQq = len(QSq)
    KCq = -(-Vq // 128)  # query-row chunks for the output projection
    NS = [(0, 512), (512, 512)]  # output D column halves

    nc = bass.Bass(trn_type="TRN2", target_bir_lowering=False, debug=False,
                   num_devices=NCORES)
    xt = nc.declare_dram_parameter("xt", [D, Vp], F32R, isOutput=False).ap()
    wqk = nc.declare_dram_parameter("wqk", [D, 1024], F32R, isOutput=False).ap()
    wv = nc.declare_dram_parameter("wv", [D, 512], F32R, isOutput=False).ap()
    wo = nc.declare_dram_parameter("wo", [512, D], BF16, isOutput=False).ap()
    biasv = nc.declare_dram_parameter("biasv", [128, KC], F32, isOutput=False).ap()
    onesd = nc.declare_dram_parameter("onesd", [128, 8], BF16, isOutput=False).ap()
    y = nc.declare_dram_parameter("y", [Vq, D], F32, isOutput=True).ap()

    with TileContext(nc) as tc:
        with tc.tile_pool(name="consts", bufs=1) as consts, \
             tc.tile_pool(name="wo", bufs=1) as wopool, \
             tc.tile_pool(name="qk", bufs=1) as qkpool, \
             tc.tile_pool(name="vp", bufs=1) as vppool, \
             tc.tile_pool(name="pt", bufs=2 * KC + 2) as ptpool, \
             tc.tile_pool(name="at", bufs=1) as atpool, \
             tc.tile_pool(name="norm", bufs=4) as npool, \
             tc.tile_pool(name="ysb", bufs=2) as ypool, \
             tc.tile_pool(name="dsc", bufs=8, space="DRAM") as dpool:

            bias_sb = consts.tile([128, KC], F32)
            ones_sb = consts.tile([128, 8], BF16)

            qk_sb = []
            vp_sb = []
            pair_mark = None
            # stage-1 inputs live only in this scope
            with tc.tile_pool(name="xsb", bufs=1) as xpool, \
                 tc.tile_pool(name="wqk", bufs=1) as wqkpool, \
                 tc.tile_pool(name="wv", bufs=1) as wvpool, \
                 tc.tile_pool(name="qkps", bufs=3, space="PSUM") as qkps, \
                 tc.tile_pool(name="vps", bufs=2, space="PSUM") as vps:
                xsb, wqk_sb = [], []
                for k in range(8):
                    wt = wqkpool.tile([128, 1024], F32R, tag=f"wqk{k}",
                                      name=f"wqk_{k}")
                    nc.sync.dma_start(out=wt[:],
                                      in_=wqk[k * 128:(k + 1) * 128, :])
                    wqk_sb.append(wt)
                    t = xpool.tile([128, Vp], F32R, tag=f"x{k}", name=f"x_{k}")
                    nc.scalar.dma_start(out=t[:],
                                        in_=xt[k * 128:(k + 1) * 128, :])
                    xsb.append(t)
                wv_sb = []
                for k in range(8):
                    wt = wvpool.tile([128, 512], F32R, tag=f"wv{k}",
                                     name=f"wv_{k}")
                    nc.scalar.dma_start(out=wt[:],
                                        in_=wv[k * 128:(k + 1) * 128, :])
                    wv_sb.append(wt)
                nc.sync.dma_start(out=bias_sb[:], in_=biasv[:])
                nc.sync.dma_start(out=ones_sb[:], in_=onesd[:])

                # ---- stage 1a: qkT[m] = (wqk[:, m*128:+128]).T @ xT ----
                # emit as (q, k) head-pair groups so heads can start early
                qk_sb.extend([None] * 8)
                pair_mark = [0] * 4
                for p in range(4):
                    for m in (p, 4 + p):
                        isq = m < 4
                        mW, mQS, mNQ, mV = ((Wq, QSq, NQq, Vq) if isq
                                            else (W, QS, NQ, Vp))
                        ps = qkps.tile([128, mNQ, 512], F32, tag="qkp",
                                       name=f"qkp_{m}")
                        for qi, (n0, nw) in enumerate(mQS):
                            for k in range(8):
                                nc.tensor.matmul(
                                    ps[:, qi, 0:nw],
                                    lhsT=wqk_sb[k][:, m * 128:(m + 1) * 128],
                                    rhs=xsb[k][:, n0:n0 + nw],
                                    start=(k == 0), stop=(k == 7))
                        qt = qkpool.tile([128, mV], F32R, tag=f"qk{m}",
                                         name=f"qk_{m}")
                        qtv = qt[:].rearrange("p (q w) -> p q w", q=mNQ)
                        nc.vector.tensor_copy(out=qtv, in_=ps[:, :, 0:mW])
                        qk_sb[m] = qt
                    pair_mark[p] = tc.cur_priority

                # ---- stage 1b: V' tiles [128, 8, 65] per key chunk ----
                for c in range(KC):
                    ps = vps.tile([128, 512], F32, tag="vpp")
                    for k in range(8):
                        nc.tensor.matmul(ps[:],
                                         lhsT=xsb[k][:, c * 128:(c + 1) * 128],
                                         rhs=wv_sb[k][:],
                                         start=(k == 0), stop=(k == 7))
                    vt = vppool.tile([128, 8, 65], BF16, tag=f"vp{c}")
                    nc.vector.tensor_copy(
                        out=vt[:, :, 0:64],
                        in_=ps[:].rearrange("p (h d) -> p h d", h=8))
                    nc.vector.tensor_copy(
                        out=vt[:, :, 64:65],
                        in_=ones_sb[:].rearrange("p (a b) -> p a b", b=1))
                    vp_sb.append(vt)

            wo_sb = []
            for j in range(4):
                wt = wopool.tile([128, 1024], BF16, tag=f"wo{j}")
                nc.sync.dma_start(out=wt[:], in_=wo[j * 128:(j + 1) * 128, :])
                wo_sb.append(wt)

            # ---- stage 2: per-head S^T -> exp -> O^T -> normalize ----
            at2 = [atpool.tile([128, Vq], BF16, tag=f"at{j}", name=f"at2_{j}")
                   for j in range(4)]
            with tc.tile_pool(name="stps", bufs=3, space="PSUM") as stps, \
                 tc.tile_pool(name="otps", bufs=1, space="PSUM") as otps, \
                 tc.tile_pool(name="odd", bufs=4) as oddpool:
                for hp in range(4):
                    qt = qk_sb[hp]
                    kt = qk_sb[4 + hp]
                    # S^T for the head pair: the even head occupies PE rows
                    # 0-63 and the odd head rows 64-127 (adjacent emission ->
                    # the two matmuls run concurrently on disjoint row groups)
                    pts = {0: [], 1: []}
                    with tc.high_priority(
                            offset=tc.cur_priority - pair_mark[hp]):
                        for c in range(KC):
                            for sub in (0, 1):
                                lo = sub * 64
                                st = stps.tile([128, NQq, 512], F32, tag="st",
                                               name=f"st_{hp}_{sub}_{c}")
                                for qi, (n0, nw) in enumerate(QSq):
                                    nc.tensor.matmul(
                                        st[:, qi, 0:nw],
                                        lhsT=kt[lo:lo + 64,
                                                c * 128:(c + 1) * 128],
                                        rhs=qt[lo:lo + 64, n0:n0 + nw],
                                        start=True, stop=True)
                                pts[sub].append(st)
                            for sub in (0, 1):
                                st = pts[sub][c]
                                pt = ptpool.tile([128, Vq], BF16, tag="pt",
                                                 name=f"pt_{hp}_{sub}_{c}")
                                nc.scalar.activation(
                                    out=pt[:].rearrange("p (q w) -> p q w",
                                                        q=NQq),
                                    in_=st[:, :, 0:Wq], func=AF.Exp,
                                    bias=bias_sb[:, c:c + 1], scale=1.0)
                                pts[sub][c] = pt
                    for sub in (0, 1):
                        h = 2 * hp + sub
                        ot = otps.tile([65, NQq, 512], F32, tag="ot",
                                       name=f"ot_{h}")
                        for c in range(KC):
                            for qi, (n0, nw) in enumerate(QSq):
                                nc.tensor.matmul(ot[:, qi, 0:nw],
                                                 lhsT=vp_sb[c][:, h, :],
                                                 rhs=pts[sub][c][:, n0:n0 + nw],
                                                 start=(c == 0),
                                                 stop=(c == KC - 1))
                        # evacuate O^T from PSUM right away so the next head's
                        # O^T can start: numerators -> osb, denom row -> rden.
                        osb = npool.tile([64, Vq], F32, tag="osb")
                        nc.vector.tensor_copy(
                            out=osb[:].rearrange("p (q w) -> p q w", q=NQq),
                            in_=ot[0:64, :, 0:Wq])
                        # 1/denom via exp(-ln d) on ACT (same table set as
                        # the softmax exp), then DRAM-bounce the row into a
                        # [64, Vq] partition-broadcast.
                        rln = npool.tile([65, Vq], F32, tag="rln")
                        nc.scalar.activation(
                            out=rln[64:65, :].rearrange(
                                "p (q w) -> p q w", q=NQq),
                            in_=ot[64:65, :, 0:Wq], func=AF.Ln)
                        rexp = npool.tile([65, Vq], F32, tag="rexp")
                        nc.scalar.activation(out=rexp[64:65, :],
                                             in_=rln[64:65, :],
                                             func=AF.Exp, scale=-1.0)
                        sc3 = dpool.tile([1, Vq], F32, tag="sc3")
                        nc.sync.dma_start(out=sc3[:], in_=rexp[64:65, :])
                        rsrc = sc3[0, :]
                        rbc = npool.tile([64, Vq], F32, tag="rbc")
                        bsrc = bass.AP(tensor=rsrc.tensor, offset=rsrc.offset,
                                       ap=[[0, 64]] + list(rsrc.ap))
                        nc.sync.dma_start(out=rbc[:], in_=bsrc)
                        if sub == 0:
                            nc.vector.tensor_mul(at2[hp][0:64, :],
                                                 osb[:], rbc[:])
                        else:
                            tmp = oddpool.tile([64, Vq], BF16, tag="odd")
                            nc.vector.tensor_mul(tmp[:], osb[:], rbc[:])
                            nc.sync.dma_start(out=at2[hp][64:128, :],
                                              in_=tmp[:])

            # ---- stage 3: y[qc] = sum_j Apair_j @ wopair_j  (K=128) ----
            with tc.tile_pool(name="yps", bufs=2, space="PSUM") as yps:
                for qc in range(KCq):
                    mw = min(128, Vq - qc * 128)
                    yp = yps.tile([128, 1024], F32, tag="yp")
                    for j in range(4):
                        for (n0, nw) in NS:
                            nc.tensor.matmul(
                                yp[0:mw, n0:n0 + nw],
                                lhsT=at2[j][:, qc * 128:qc * 128 + mw],
                                rhs=wo_sb[j][:, n0:n0 + nw],
                                start=(j == 0), stop=(j == 3))
                    ysb = ypool.tile([128, 1024], F32, tag="ysb")
                    nc.vector.tensor_copy(out=ysb[0:mw, :], in_=yp[0:mw, :])
                    nc.scalar.dma_start(out=y[qc * 128:qc * 128 + mw, :],
                                        in_=ysb[0:mw, :])
    return nc


def kernel(x, mask, w_qkv, w_out, b_out):
    _install_patches()
    from concourse.bass_utils import run_bass_kernel_spmd

    x = np.asarray(x, dtype=np.float32)
    mask = np.asarray(mask, dtype=np.float32)
    w_qkv = np.asarray(w_qkv, dtype=np.float32)
    w_out = np.asarray(w_out, dtype=np.float32)
    b_out = np.asarray(b_out, dtype=np.float32)

    idx = [np.nonzero(mask[b] != 0.0)[0] for b in range(B)]
    nv = [len(i) for i in idx]
    Vp = max(128, int(-(-max(nv) // 128)) * 128)
    Vq = max(128, int(-(-max(nv) // 32)) * 32)
    if max(nv) == 0:
        return np.broadcast_to(b_out, (B, N, D)).astype(np.float32).copy()

    scale = float(DH) ** -0.5
    G = 512  # features per head-group
    wqk_g, wv_g, wo_g = [], [], []
    for g in range(2):
        wq = w_qkv[:, g * G:(g + 1) * G] * scale
        wk = w_qkv[:, 1024 + g * G:1024 + (g + 1) * G]
        wqk_g.append(np.ascontiguousarray(np.concatenate([wq, wk], axis=1)))
        wv_g.append(np.ascontiguousarray(w_qkv[:, 2048 + g * G:2048 + (g + 1) * G]))
        wo_g.append(np.ascontiguousarray(w_out[g * G:(g + 1) * G, :]))

    import ml_dtypes
    bf16 = ml_dtypes.bfloat16
    wo_g = [w.astype(bf16) for w in wo_g]
    xt_b, bias_b = [], []
    for b in range(B):
        pad = Vp - nv[b]
        idxp = np.concatenate([idx[b], np.zeros(pad, dtype=np.int64)])
        xg = x[b][idxp, :]
        xt_b.append(np.ascontiguousarray(xg.T))
        bv = np.concatenate([
            np.zeros(nv[b], dtype=np.float32),
            np.full(pad, PAD_BIAS, dtype=np.float32)])
        bias_b.append(np.ascontiguousarray(bv.reshape(-1, 128).T))
    ones = np.ones((128, 8), dtype=ml_dtypes.bfloat16)

    nc = _build_program(Vp, Vq)
    in_maps = []
    for core in range(NCORES):
        b, g = core // 2, core % 2
        in_maps.append({
            "xt": xt_b[b], "wqk": wqk_g[g], "wv": wv_g[g], "wo": wo_g[g],
            "biasv": bias_b[b], "onesd": ones,
        })

    trace = bool(os.environ.get("BASSK_TRACE"))
    if trace:
        _install_profile_hook()
    res = run_bass_kernel_spmd(nc, in_maps, list(range(NCORES)), trace=trace)
    global last_exec_time_ns
    last_exec_time_ns = res.exec_time_ns

    out = np.zeros((B, N, D), dtype=np.float32)
    for b in range(B):
        yb = res.results[2 * b]["y"] + res.results[2 * b + 1]["y"]
        out[b][idx[b]] = yb[:nv[b]]
    out += b_out
    return out


last_exec_time_ns = None


def _install_profile_hook():
    import types
    import antenv
    if 'antenv.axon_hooks' in sys.modules:
        return
    import trn_agent_boot.trn_boot as tb
    _hook = tb._ntff_profile_via_ctypes('/opt/axon/libaxon_pjrt.so')
    mod = types.ModuleType('antenv.axon_hooks')
    mod.get_axon_ntff_profile_hook = lambda: _hook
    mod.set_axon_ntff_profile_hook = lambda h: None
    sys.modules['antenv.axon_hooks'] = mod
    antenv.axon_hooks = mod
    bass_utils.upload_artifacts = lambda tmpdir: "local://skipped"


# revision 27
# speedup vs baseline: 1.0197x; 1.0197x over previous
"""Masked multi-head attention block (B=4, N=1024, D=1024, H=16, DH=64) on 8
Trainium2 NeuronCores.

Sharding: core (b, g) = 2*b + g handles batch b and head-group g (8 of 16
heads). Each core computes qkv projections for its heads, attention, and its
partial output projection; the host sums the two head-group partials per batch.

Mask handling: tokens with mask==0 neither attend nor are attended to (the
reference sets fully-masked rows to zero and -inf-masks columns). The host
gathers only the valid tokens per batch (padded to a multiple of 128 with
key-bias -30000 on the pad), so the device computes a dense unmasked attention
over ~half the sequence; invalid token rows of the output are b_out.

Device layout (per core, Vp = padded valid token count):
  xT   [D, Vp]      gathered tokens, transposed (host prep)
  qkT  [1024, Vp]   = [Q^T; K^T] for 8 heads, via lhsT=wqk chunks, rhs=xT
  V'   [Vp, 8, 65]  values per head + ones column (softmax denominator)
  S^T  [Vp, Vp]/head = K_chunk @ Q^T; exp on ACT with per-partition pad bias
  O^T  [65, Vp]/head = V'^T @ P^T accumulated over key chunks (row 64 = denom)
  A^T  [64, Vp]/head = O^T * (1/denom) broadcast (DRAM-bounce broadcast DMA)
  y    [Vp, D]      = sum_h A_h @ w_out_h  (K=64 accumulating matmuls)
"""
import json
import os
import sys

import numpy as np

sys.path.insert(0, "/opt/trn_rl_repo")

import concourse.bass as bass
import concourse.mybir as mybir
from concourse.tile import TileContext
from concourse import bass_utils

F32 = mybir.dt.float32
F32R = mybir.dt.float32r
BF16 = mybir.dt.bfloat16
AF = mybir.ActivationFunctionType

B, N, D, H, DH = 4, 1024, 1024, 16, 64
NCORES = 8
PAD_BIAS = -30000.0


def _install_patches():
    """The walrus build in this container accepts only one semaphore wait per
    instruction; hoist extra waits onto same-engine NoOps in the BIR json."""
    if getattr(bass.Bass, "_split_waits_patched", False):
        return
    orig = bass.Bass.to_json_bytes

    def to_json_bytes_split(self, *a, **k):
        j = json.loads(orig(self, *a, **k))
        for fn in j.get("functions", []):
            for bb in fn.get("blocks", []):
                out = []
                for ins in bb.get("instructions", []):
                    si = ins.get("sync_info") or {}
                    waits = si.get("on_wait") or []
                    if len(waits) > 1:
                        for i, w in enumerate(waits[:-1]):
                            out.append({
                                "debug": ins.get("debug", 0),
                                "engine": ins["engine"],
                                "ins": [],
                                "name": f"{ins['name']}_sw{i}",
                                "opcode": "NoOp",
                                "outs": [],
                                "text_hint": "splitw",
                                "sync_info": {"on_update": [], "on_wait": [w]},
                            })
                        si["on_wait"] = [waits[-1]]
                    out.append(ins)
                bb["instructions"] = out
        return json.dumps(j).encode()

    bass.Bass.to_json_bytes = to_json_bytes_split

    def _drain_and_barrier(self, tick_clock, wait_clock):
        import re as _re
        import bass_rust as _br
        from concourse.vector_clock import ScopedClock as _SC
        gc = tick_clock.global_clock
        comps = eval(_re.match(r"VectorClock\((\[.*\])\)", repr(gc)).group(1))
        for i, v in enumerate(comps):
            if v <= 0:
                continue
            sub = [0] * len(comps)
            sub[i] = v
            nop = self.nc.sync.nop(nofuse=True, hint="final_wait")
            wait_clock.add_sem_waits(nop.ins, _SC({None: _br.VectorClock(sub)}))
        self.nc.sync.drain()
        self.nc.all_engine_barrier()
        assert self.sems is not None
        popped = self.nc._tile_sem_poison_stack.pop()
        assert popped is self._sem_poison
        self.nc.clear_and_free_semaphores(list(self.sems.allocated().values()))

    TileContext._drain_and_barrier = _drain_and_barrier
    bass.Bass._split_waits_patched = True


def _build_program(Vp, Vq):
    KC = Vp // 128
    # key-side and query-side column slices: each slice gets its own PSUM
    # bank (matmul output must not cross a bank) and stays >= 256 wide.
    W = Vp if Vp <= 512 else Vp // 2
    QS = [(i * W, W) for i in range(Vp // W)]
    NQ = len(QS)
    Wq = Vq if Vq <= 512 else Vq // 2
    QSq = [(i * Wq, Wq) for i in range(Vq // Wq)]
    NQq = len(QSq)
    KCq = -(-Vq // 128)  # query-row chunks for the output projection
    NS = [(0, 512), (512, 512)]  # output D column halves

    nc = bass.Bass(trn_type="TRN2", target_bir_lowering=False, debug=False,
                   num_devices=NCORES)
    xt = nc.declare_dram_parameter("xt", [D, Vp], F32R, isOutput=False).ap()
    wqk = nc.declare_dram_parameter("wqk", [D, 1024], F32R, isOutput=False).ap()
    wv = nc.declare_dram_parameter("wv", [D, 512], F32R, isOutput=False).ap()
    wo = nc.declare_dram_parameter("wo", [512, D], BF16, isOutput=False).ap()
    biasv = nc.declare_dram_parameter("biasv", [128, KC], F32, isOutput=False).ap()
    onesd = nc.declare_dram_parameter("onesd", [128, 8], BF16, isOutput=False).ap()
    y = nc.declare_dram_parameter("y", [Vq, D], F32, isOutput=True).ap()

    with TileContext(nc) as tc:
        with tc.tile_pool(name="consts", bufs=1) as consts, \
             tc.tile_pool(name="wo", bufs=1) as wopool, \
             tc.tile_pool(name="qk", bufs=1) as qkpool, \
             tc.tile_pool(name="vp", bufs=1) as vppool, \
             tc.tile_pool(name="pt", bufs=2 * KC + 5) as ptpool, \
             tc.tile_pool(name="at", bufs=1) as atpool, \
             tc.tile_pool(name="norm", bufs=4) as npool, \
             tc.tile_pool(name="ysb", bufs=2) as ypool, \
             tc.tile_pool(name="dsc", bufs=8, space="DRAM") as dpool:

            bias_sb = consts.tile([128, KC], F32)
            ones_sb = consts.tile([128, 8], BF16)

            qk_sb = []
            vp_sb = []
            pair_mark = None
            # stage-1 inputs live only in this scope
            with tc.tile_pool(name="xsb", bufs=1) as xpool, \
                 tc.tile_pool(name="wqk", bufs=1) as wqkpool, \
                 tc.tile_pool(name="wv", bufs=1) as wvpool, \
                 tc.tile_pool(name="qkps", bufs=3, space="PSUM") as qkps, \
                 tc.tile_pool(name="vps", bufs=2, space="PSUM") as vps:
                xsb, wqk_sb = [], []
                for k in range(8):
                    wt = wqkpool.tile([128, 1024], F32R, tag=f"wqk{k}",
                                      name=f"wqk_{k}")
                    nc.sync.dma_start(out=wt[:],
                                      in_=wqk[k * 128:(k + 1) * 128, :])
                    wqk_sb.append(wt)
                    t = xpool.tile([128, Vp], F32R, tag=f"x{k}", name=f"x_{k}")
                    nc.scalar.dma_start(out=t[:],
                                        in_=xt[k * 128:(k + 1) * 128, :])
                    xsb.append(t)
                wv_sb = []
                for k in range(8):
                    wt = wvpool.tile([128, 512], F32R, tag=f"wv{k}",
                                     name=f"wv_{k}")
                    nc.scalar.dma_start(out=wt[:],
                                        in_=wv[k * 128:(k + 1) * 128, :])
                    wv_sb.append(wt)
                nc.sync.dma_start(out=bias_sb[:], in_=biasv[:])
                nc.sync.dma_start(out=ones_sb[:], in_=onesd[:])

                # ---- stage 1a: qkT[m] = (wqk[:, m*128:+128]).T @ xT ----
                # emit as (q, k) head-pair groups so heads can start early
                qk_sb.extend([None] * 8)
                pair_mark = [0] * 4
                for p in range(4):
                    for m in (p, 4 + p):
                        isq = m < 4
                        mW, mQS, mNQ, mV = ((Wq, QSq, NQq, Vq) if isq
                                            else (W, QS, NQ, Vp))
                        ps = qkps.tile([128, mNQ, 512], F32, tag="qkp",
                                       name=f"qkp_{m}")
                        for qi, (n0, nw) in enumerate(mQS):
                            for k in range(8):
                                nc.tensor.matmul(
                                    ps[:, qi, 0:nw],
                                    lhsT=wqk_sb[k][:, m * 128:(m + 1) * 128],
                                    rhs=xsb[k][:, n0:n0 + nw],
                                    start=(k == 0), stop=(k == 7))
                        qt = qkpool.tile([128, mV], F32R, tag=f"qk{m}",
                                         name=f"qk_{m}")
                        qtv = qt[:].rearrange("p (q w) -> p q w", q=mNQ)
                        nc.vector.tensor_copy(out=qtv, in_=ps[:, :, 0:mW])
                        qk_sb[m] = qt
                    pair_mark[p] = tc.cur_priority

                # ---- stage 1b: V' tiles [128, 8, 65] per key chunk ----
                for c in range(KC):
                    ps = vps.tile([128, 512], F32, tag="vpp")
                    for k in range(8):
                        nc.tensor.matmul(ps[:],
                                         lhsT=xsb[k][:, c * 128:(c + 1) * 128],
                                         rhs=wv_sb[k][:],
                                         start=(k == 0), stop=(k == 7))
                    vt = vppool.tile([128, 8, 65], BF16, tag=f"vp{c}")
                    nc.vector.tensor_copy(
                        out=vt[:, :, 0:64],
                        in_=ps[:].rearrange("p (h d) -> p h d", h=8))
                    nc.vector.tensor_copy(
                        out=vt[:, :, 64:65],
                        in_=ones_sb[:].rearrange("p (a b) -> p a b", b=1))
                    vp_sb.append(vt)

            wo_sb = []
            for j in range(4):
                wt = wopool.tile([128, 1024], BF16, tag=f"wo{j}")
                nc.sync.dma_start(out=wt[:], in_=wo[j * 128:(j + 1) * 128, :])
                wo_sb.append(wt)

            # ---- stage 2: per-head S^T -> exp -> O^T -> normalize ----
            at2 = [atpool.tile([128, Vq], BF16, tag=f"at{j}", name=f"at2_{j}")
                   for j in range(4)]
            with tc.tile_pool(name="stps", bufs=3, space="PSUM") as stps, \
                 tc.tile_pool(name="otps", bufs=1, space="PSUM") as otps, \
                 tc.tile_pool(name="odd", bufs=4) as oddpool:
                for hp in range(4):
                    qt = qk_sb[hp]
                    kt = qk_sb[4 + hp]
                    # S^T for the head pair: the even head occupies PE rows
                    # 0-63 and the odd head rows 64-127 (adjacent emission ->
                    # the two matmuls run concurrently on disjoint row groups)
                    pts = {0: [], 1: []}
                    with tc.high_priority(
                            offset=tc.cur_priority - pair_mark[hp]):
                        for c in range(KC):
                            for sub in (0, 1):
                                lo = sub * 64
                                st = stps.tile([128, NQq, 512], F32, tag="st",
                                               name=f"st_{hp}_{sub}_{c}")
                                for qi, (n0, nw) in enumerate(QSq):
                                    nc.tensor.matmul(
                                        st[:, qi, 0:nw],
                                        lhsT=kt[lo:lo + 64,
                                                c * 128:(c + 1) * 128],
                                        rhs=qt[lo:lo + 64, n0:n0 + nw],
                                        start=True, stop=True)
                                pts[sub].append(st)
                            for sub in (0, 1):
                                st = pts[sub][c]
                                pt = ptpool.tile([128, Vq], BF16, tag="pt",
                                                 name=f"pt_{hp}_{sub}_{c}")
                                nc.scalar.activation(
                                    out=pt[:].rearrange("p (q w) -> p q w",
                                                        q=NQq),
                                    in_=st[:, :, 0:Wq], func=AF.Exp,
                                    bias=bias_sb[:, c:c + 1], scale=1.0)
                                pts[sub][c] = pt
                    for sub in (0, 1):
                        h = 2 * hp + sub
                        ot = otps.tile([65, NQq, 512], F32, tag="ot",
                                       name=f"ot_{h}")
                        for c in range(KC):
                            for qi, (n0, nw) in enumerate(QSq):
                                nc.tensor.matmul(ot[:, qi, 0:nw],
                                                 lhsT=vp_sb[c][:, h, :],
                                                 rhs=pts[sub][c][:, n0:n0 + nw],
                                                 start=(c == 0),
                                                 stop=(c == KC - 1))
                        # evacuate O^T from PSUM right away so the next head's
                        # O^T can start: numerators -> osb, denom row -> rden.
                        osb = npool.tile([64, Vq], F32, tag="osb")
                        nc.vector.tensor_copy(
                            out=osb[:].rearrange("p (q w) -> p q w", q=NQq),
                            in_=ot[0:64, :, 0:Wq])
                        # 1/denom via exp(-ln d) on ACT (same table set as
                        # the softmax exp), then DRAM-bounce the row into a
                        # [64, Vq] partition-broadcast.
                        rln = npool.tile([65, Vq], F32, tag="rln")
                        nc.scalar.activation(
                            out=rln[64:65, :].rearrange(
                                "p (q w) -> p q w", q=NQq),
                            in_=ot[64:65, :, 0:Wq], func=AF.Ln)
                        rexp = npool.tile([65, Vq], F32, tag="rexp")
                        nc.scalar.activation(out=rexp[64:65, :],
                                             in_=rln[64:65, :],
                                             func=AF.Exp, scale=-1.0)
                        sc3 = dpool.tile([1, Vq], F32, tag="sc3")
                        nc.sync.dma_start(out=sc3[:], in_=rexp[64:65, :])
                        rsrc = sc3[0, :]
                        rbc = npool.tile([64, Vq], F32, tag="rbc")
                        bsrc = bass.AP(tensor=rsrc.tensor, offset=rsrc.offset,
                                       ap=[[0, 64]] + list(rsrc.ap))
                        nc.sync.dma_start(out=rbc[:], in_=bsrc)
                        if sub == 0:
                            nc.vector.tensor_mul(at2[hp][0:64, :],
                                                 osb[:], rbc[:])
                        else:
                            tmp = oddpool.tile([64, Vq], BF16, tag="odd")
                            nc.vector.tensor_mul(tmp[:], osb[:], rbc[:])
                            nc.sync.dma_start(out=at2[hp][64:128, :],
                                              in_=tmp[:])

            # ---- stage 3: y[qc] = sum_j Apair_j @ wopair_j  (K=128) ----
            with tc.tile_pool(name="yps", bufs=2, space="PSUM") as yps:
                for qc in range(KCq):
                    mw = min(128, Vq - qc * 128)
                    yp = yps.tile([128, 1024], F32, tag="yp")
                    for j in range(4):
                        for (n0, nw) in NS:
                            nc.tensor.matmul(
                                yp[0:mw, n0:n0 + nw],
                                lhsT=at2[j][:, qc * 128:qc * 128 + mw],
                                rhs=wo_sb[j][:, n0:n0 + nw],
                                start=(j == 0), stop=(j == 3))
                    ysb = ypool.tile([128, 1024], F32, tag="ysb")
                    nc.vector.tensor_copy(out=ysb[0:mw, :], in_=yp[0:mw, :])
                    nc.scalar.dma_start(out=y[qc * 128:qc * 128 + mw, :],
                                        in_=ysb[0:mw, :])
    return nc


def kernel(x, mask, w_qkv, w_out, b_out):
    _install_patches()
    from concourse.bass_utils import run_bass_kernel_spmd

    x = np.asarray(x, dtype=np.float32)
    mask = np.asarray(mask, dtype=np.float32)
    w_qkv = np.asarray(w_qkv, dtype=np.float32)
    w_out = np.asarray(w_out, dtype=np.float32)
    b_out = np.asarray(b_out, dtype=np.float32)

    idx = [np.nonzero(mask[b] != 0.0)[0] for b in range(B)]
    nv = [len(i) for i in idx]
    Vp = max(128, int(-(-max(nv) // 128)) * 128)
    Vq = max(128, int(-(-max(nv) // 32)) * 32)
    if max(nv) == 0:
        return np.broadcast_to(b_out, (B, N, D)).astype(np.float32).copy()

    scale = float(DH) ** -0.5
    G = 512  # features per head-group
    wqk_g, wv_g, wo_g = [], [], []
    for g in range(2):
        wq = w_qkv[:, g * G:(g + 1) * G] * scale
        wk = w_qkv[:, 1024 + g * G:1024 + (g + 1) * G]
        wqk_g.append(np.ascontiguousarray(np.concatenate([wq, wk], axis=1)))
        wv_g.append(np.ascontiguousarray(w_qkv[:, 2048 + g * G:2048 + (g + 1) * G]))
        wo_g.append(np.ascontiguousarray(w_out[g * G:(g + 1) * G, :]))

    import ml_dtypes
    bf16 = ml_dtypes.bfloat16
    wo_g = [w.astype(bf16) for w in wo_g]
    xt_b, bias_b = [], []
    for b in range(B):
        pad = Vp - nv[b]
        idxp = np.concatenate([idx[b], np.zeros(pad, dtype=np.int64)])
        xg = x[b][idxp, :]
        xt_b.append(np.ascontiguousarray(xg.T))
        bv = np.concatenate([
            np.zeros(nv[b], dtype=np.float32),
            np.full(pad, PAD_BIAS, dtype=np.float32)])
        bias_b.append(np.ascontiguousarray(bv.reshape(-1, 128).T))
    ones = np.ones((128, 8), dtype=ml_dtypes.bfloat16)

    nc = _build_program(Vp, Vq)
    in_maps = []
    for core in range(NCORES):
        b, g = core // 2, core % 2
        in_maps.append({
            "xt": xt_b[b], "wqk": wqk_g[g], "wv": wv_g[g], "wo": wo_g[g],
            "biasv": bias_b[b], "onesd": ones,
        })

    trace = bool(os.environ.get("BASSK_TRACE"))
    if trace:
        _install_profile_hook()
    res = run_bass_kernel_spmd(nc, in_maps, list(range(NCORES)), trace=trace)
    global last_exec_time_ns
    last_exec_time_ns = res.exec_time_ns

    out = np.zeros((B, N, D), dtype=np.float32)
    for b in range(B):
        yb = res.results[2 * b]["y"] + res.results[2 * b + 1]["y"]
        out[b][idx[b]] = yb[:nv[b]]
    out += b_out
    return out


last_exec_time_ns = None


def _install_profile_hook():
    import types
    import antenv
    if 'antenv.axon_hooks' in sys.modules:
        return
    import trn_agent_boot.trn_boot as tb
    _hook = tb._ntff_profile_via_ctypes('/opt/axon/libaxon_pjrt.so')
    mod = types.ModuleType('antenv.axon_hooks')
    mod.get_axon_ntff_profile_hook = lambda: _hook
    mod.set_axon_ntff_profile_hook = lambda h: None
    sys.modules['antenv.axon_hooks'] = mod
    antenv.axon_hooks = mod
    bass_utils.upload_artifacts = lambda tmpdir: "local://skipped"
